# revision 13
# baseline (speedup 1.0000x reference)
"""Trainium2 Bass kernel for single-head attention with row-major K-reshape.

Reference computation (per batch b):
    Q = x @ W_Q.T ; K = x @ W_K.T ; V = x @ W_V.T          # [S, D]
    K_r = K.reshape(D, S)          # row-major reshape, NOT a transpose
    scores = Q @ K_r / D
    out = softmax(scores, -1) @ V

Shapes: B=4, S=2048, D=1024, f32.

Sharding: 8 cores = (batch b in 0..3) x (pair-rank h in 0..1).  Core (b, h)
computes out[b, h*QB:(h+1)*QB, :].  K_r and V for batch b are computed
cooperatively by the pair (b,0)/(b,1) — each core builds one half and the
halves are exchanged with pair-wise AllGathers (DRAM bounce, fragments
laid out partition-major so each gather pulls back into SBUF with just
two wide DMAs):

  K_r half:  with S == 2*D the row-major reshape gives
                 K_r[m, g*D + c] = K[2m + g, c]
             so rank g's half is  x[g::2, :] @ W_K.T  — the parity-g rows
             of x ("xp").  Fragment g is exactly global columns
             [g*D, (g+1)*D) of K_r.
  V half:    rank g computes V rows [g*QB, (g+1)*QB) = xq @ W_V.T — the
             same rows as its query block ("xq").  The fragment also
             carries the rank's exact bf16 column-sum row (see below),
             byte-packed into a spare column block.

A zero-byte dummy AllGather is issued at kernel start so the collective
firmware pays its ~20us first-collective setup while the operand DMAs
are still streaming.

Host-side packing (layout/dtype prep only, numpy, once per call): all
operands are shipped pre-transposed AND pre-tiled as [P, NDT, cols]
(contraction dim split over partition x tile-index) so each one loads
with four wide DMAs and TensorE does zero transposes: xqT bf16 (V-proj
lhsT), xqT8 fp8 (Q-proj rhs), xpT8 fp8 (K-proj lhsT), wqT8/wkT8 fp8
scaled by 16, wvT bf16.

Precision plan (tolerance is 2e-2 relative to max|out|; this plan
CPU-simulates to 5.9e-3; the bf16 baseline measured 4.7e-3):
  fp8e4 + perf_mode=DoubleRow (2 k-tiles per instruction, 216 ns per
  N=512 matmul — true 2x) for Q-proj, K-proj, scores AND the attn @ V
  matmul.  W_Q/W_K carry a x16 pre-scale (folded into the exp).  The
  V projection stays bf16 (V errors pass straight to the output; fp8
  there measures 4.5e-2).  The attn @ V matmul survives fp8 via an
  offset trick: with near-uniform attention, e = exp(s) ~= 1, so
      out = (eps @ V8 + colsum_V) / (rowsum(eps) + S),   eps = f8(e - 1)
  where eps carries fp8 noise only on the +-0.15-magnitude deviation
  (attenuated ~1/sqrt(S) in the output) and colsum_V = sum_j V[j,:] is
  computed exactly per fragment half from the f32 PSUM V values
  (ones-matmul; colsum from fp8 V would measure 2.6e-2), exchanged as
  bf16 inside the V fragment, and added via a DVE broadcast add on the
  output drain.

Dataflow per core (TensorE matmul computes out[M,N] = lhsT[K,M].T @ rhs[K,N],
contraction over the partition dim; inner loops reuse each stationary
lhsT across both output chunks to halve LDWEIGHTS traffic):
    KRfrag[m, c] = lhsT=xpT8[:, pair, m], rhs=wkT8[:, pair, c]   (fp8 DR)
    Vfrag[s', c] = lhsT=xqT[:, dt, s'],   rhs=wvT[:, dt, c]      (bf16,
                   drained to fp8 for the gather + transient bf16 for
                   the per-half colsum ones-matmuls)
    QT8[m, i]    = lhsT=wqT8[:, pair, m], rhs=xqT8[:, pair, i]   (fp8 DR)
    KR8/V8       = one pair AllGather each, pulled into SBUF with two
                   wide DMAs on the gpsimd DGE ring
    ST[j, i]     = lhsT=KR8[:, pair, j],  rhs=QT8[:, pair, i]    (fp8 DR)
    et           = exp(ST / (D*256))      (ACT, psum->bf16 ring buffer)
    eps[j, i]    = et - 1 -> fp8          (DVE)
    colsumP      = both halves' colsum rows added (DVE) and broadcast
                   to all partitions with two K=1 matmuls
    O[i, c]      = lhsT=eps[:, pair, i],  rhs=V8[:, pair, c]     (fp8 DR)
    rsum[i, 1]   = lhsT=eps pair (shared), rhs=ones8 [P,2,1]  (fp8 DR,
                   fused into the O loop -> its weight load is free)
    out          = (O + colsumP) * (1 / (rsum + S))   (DVE)
"""

from contextlib import ExitStack

import ml_dtypes
import numpy as np

import concourse.tile as tile
from concourse import bacc, mybir
from concourse.bass_utils import run_bass_kernel_spmd

F32 = mybir.dt.float32
BF16 = mybir.dt.bfloat16
F8 = mybir.dt.float8e4
NP_BF16 = ml_dtypes.bfloat16
NP_F8 = ml_dtypes.float8_e4m3fn
P = 128
WS = 16.0  # fp8 pre-scale for W_Q / W_K


def build_attention(nc, S=2048, D=1024, QB=1024, n_cores=8):
    """Emit the per-core attention program into `nc`. Requires S == 2*D == 2*QB."""
    assert S == 2 * D and QB == D and D % P == 0
    NST = S // P        # seq tiles (16)
    NDT = D // P        # d_model tiles (8)
    NQT = QB // P       # query tiles for this core (8)
    NPR = NDT // 2      # DoubleRow k-tile pairs, d_model contraction (4)
    NPS = NST // 2      # DoubleRow k-tile pairs, seq contraction (8)
    NC = min(512, D)    # matmul free-dim chunk (one PSUM bank of f32)
    NCH_D = D // NC     # chunks over output channels (2)
    NCH_Q = QB // NC    # chunks over queries (2)
    EXP = mybir.ActivationFunctionType.Exp
    DR = mybir.MatmulPerfMode.DoubleRow
    groups = [[2 * b, 2 * b + 1] for b in range(n_cores // 2)]

    xqt_ap = nc.dram_tensor("xqt", [P, NDT, QB], BF16, kind="ExternalInput").ap()
    xqt8_ap = nc.dram_tensor("xqt8", [P, NDT, QB], F8, kind="ExternalInput").ap()
    xpt8_ap = nc.dram_tensor("xpt8", [P, NDT, D], F8, kind="ExternalInput").ap()
    wqt_ap = nc.dram_tensor("wqt", [P, NDT, D], F8, kind="ExternalInput").ap()
    wkt_ap = nc.dram_tensor("wkt", [P, NDT, D], F8, kind="ExternalInput").ap()
    wvt_ap = nc.dram_tensor("wvt", [P, NDT, D], BF16, kind="ExternalInput").ap()
    out_ap = nc.dram_tensor("out", [QB, D], F32, kind="ExternalOutput").ap()

    with tile.TileContext(nc) as tc, ExitStack() as ctx:
        const_pool = ctx.enter_context(tc.tile_pool(name="const", bufs=1))
        qt_pool = ctx.enter_context(tc.tile_pool(name="qt", bufs=1))
        kr_pool = ctx.enter_context(tc.tile_pool(name="kr", bufs=1))
        v8_pool = ctx.enter_context(tc.tile_pool(name="v8", bufs=1))
        eps_pool = ctx.enter_context(tc.tile_pool(name="eps", bufs=1))
        dram = ctx.enter_context(tc.tile_pool(name="dram", bufs=1, space="DRAM"))
        psum_mm = ctx.enter_context(tc.tile_pool(name="psum_mm", bufs=4, space="PSUM"))

        ones = const_pool.tile([P, 1], BF16)        # colsum stationary
        nc.vector.memset(ones, 1.0)
        ones1 = const_pool.tile([1, P], BF16)       # K=1 broadcast stationary
        nc.vector.memset(ones1, 1.0)
        ones8 = const_pool.tile([P, 2, 1], F8)      # DR rowsum rhs
        nc.vector.memset(ones8, 1.0)

        QT8 = qt_pool.tile([P, NDT, QB], F8, name="QT8")
        KR8 = kr_pool.tile([P, NDT, S], F8, name="KR8")
        V8 = v8_pool.tile([P, NST, D], F8, name="V8")
        EPS = eps_pool.tile([P, NST, QB], F8, name="EPS")

        # DRAM bounce buffers, partition-major.  The V fragment has one
        # spare column block; its first two partition rows carry the
        # rank's exact bf16 colsum row (2*D fp8 bytes == D bf16 values).
        kr_frag = dram.tile([P, NDT * D], F8, name="kr_frag")
        kr_gath = dram.tile([2, P, NDT * D], F8, name="kr_gath")
        v_frag = dram.tile([P, (NQT + 1) * D], F8, name="v_frag")
        v_gath = dram.tile([2, P, (NQT + 1) * D], F8, name="v_gath")

        with tc.tile_pool(name="xt", bufs=1) as xt_pool, \
                tc.tile_pool(name="wt", bufs=1) as wt_pool, \
                tc.tile_pool(name="frag", bufs=2) as frag_pool, \
                tc.tile_pool(name="csp", bufs=1) as csp_pool, \
                tc.tile_pool(name="psum_cs", bufs=2, space="PSUM") as psum_cs:

            xpT8 = xt_pool.tile([P, NDT, D], F8, tag="xpT8", name="xpT8")
            wkT8 = wt_pool.tile([P, NDT, D], F8, tag="wkT8", name="wkT8")
            xqT = xt_pool.tile([P, NDT, QB], BF16, tag="xqT", name="xqT")
            xqT8 = xt_pool.tile([P, NDT, QB], F8, tag="xqT8", name="xqT8")
            wvT = wt_pool.tile([P, NDT, D], BF16, tag="wvT", name="wvT")
            wqT8 = wt_pool.tile([P, NDT, D], F8, tag="wqT8", name="wqT8")

            def load3d(src_ap, dst3, eng):
                # pre-tiled operand: 4 wide DMAs across queues
                for t in range(NPR):
                    eng.dma_start(out=dst3[:, 2 * t:2 * t + 2, :],
                                  in_=src_ap[:, 2 * t:2 * t + 2, :])

            # All operand loads are pushed before any data-dependent
            # descriptor: the DGE rings are in-order, so a paced fragment
            # write ahead of a load would stall that load's descriptor.
            # K-path operands (which gate the first matmul) go first.
            load3d(xpt8_ap, xpT8, nc.scalar)
            load3d(wkt_ap, wkT8, nc.sync)
            load3d(xqt_ap, xqT, nc.sync)
            load3d(wvt_ap, wvT, nc.scalar)
            load3d(xqt8_ap, xqT8, nc.sync)
            load3d(wqt_ap, wqT8, nc.scalar)

            # ---- K_r half first: its AllGather hides under the V half ----
            for mt in range(NDT):
                kf = frag_pool.tile([P, D], F8, tag="kf", name="kf")
                pms = [psum_mm.tile([P, NC], F32, tag="pm", name=f"pm{i}") for i in range(NCH_D)]
                for t in range(NPR):
                    for cch in range(NCH_D):
                        nc.tensor.matmul(
                            pms[cch][:],
                            xpT8[:, 2 * t:2 * t + 2, mt * P:(mt + 1) * P],
                            wkT8[:, 2 * t:2 * t + 2, cch * NC:(cch + 1) * NC],
                            start=(t == 0), stop=(t == NPR - 1), perf_mode=DR,
                        )
                for cch in range(NCH_D):
                    nc.scalar.copy(kf[:, cch * NC:(cch + 1) * NC], pms[cch][:])
                nc.scalar.dma_start(out=kr_frag[:, mt * D:(mt + 1) * D], in_=kf[:])
            nc.gpsimd.collective_compute(
                "AllGather", mybir.AluOpType.bypass, replica_groups=groups,
                ins=[kr_frag.opt()], outs=[kr_gath.opt()],
            )
            for g in range(2):
                nc.gpsimd.dma_start(
                    out=KR8[:, :, g * D:(g + 1) * D].opt(keep_dims={0, 1}),
                    in_=kr_gath[g],
                )

            # ---- V half: Vfrag[st] = xq @ W_V.T in bf16; fp8 for the
            # ---- gather plus exact bf16 colsum partial over this half.
            pcs = [psum_cs.tile([1, NC], F32, tag="pc", name=f"pc{i}") for i in range(NCH_D)]
            for st in range(NQT):
                vf8 = frag_pool.tile([P, D], F8, tag="vf8", name="vf8")
                vf16 = frag_pool.tile([P, D], BF16, tag="vf16", name="vf16", bufs=3)
                pms = [psum_mm.tile([P, NC], F32, tag="pm", name=f"pm{i}") for i in range(NCH_D)]
                for dt in range(NDT):
                    for cch in range(NCH_D):
                        nc.tensor.matmul(
                            pms[cch][:],
                            xqT[:, dt, st * P:(st + 1) * P],
                            wvT[:, dt, cch * NC:(cch + 1) * NC],
                            start=(dt == 0), stop=(dt == NDT - 1),
                        )
                for cch in range(NCH_D):
                    nc.vector.tensor_copy(vf8[:, cch * NC:(cch + 1) * NC], pms[cch][:])
                    nc.vector.tensor_copy(vf16[:, cch * NC:(cch + 1) * NC], pms[cch][:])
                    nc.tensor.matmul(
                        pcs[cch][:], ones[:], vf16[:, cch * NC:(cch + 1) * NC],
                        start=(st == 0), stop=(st == NQT - 1),
                    )
                nc.sync.dma_start(out=v_frag[:, st * D:(st + 1) * D], in_=vf8[:])
            # pack this half's bf16 colsum row into the spare column block
            cs_own = csp_pool.tile([1, D], BF16, name="cs_own")
            for cch in range(NCH_D):
                nc.scalar.copy(cs_own[:, cch * NC:(cch + 1) * NC], pcs[cch][:])
            nc.scalar.dma_start(
                out=v_frag[0:2, NQT * D:(NQT + 1) * D],
                in_=cs_own.bitcast(F8)[:],
            )
            nc.gpsimd.collective_compute(
                "AllGather", mybir.AluOpType.bypass, replica_groups=groups,
                ins=[v_frag.opt()], outs=[v_gath.opt()],
            )
            for g in range(2):
                nc.gpsimd.dma_start(
                    out=V8[:, g * NQT:(g + 1) * NQT, :].opt(keep_dims={0, 1}),
                    in_=v_gath[g][:, 0:NQT * D],
                )

            # ---- QT projection (fp8 DoubleRow; KR/V gathers in flight) ----
            for mt in range(NDT):
                pms = [psum_mm.tile([P, NC], F32, tag="pm", name=f"pm{i}") for i in range(NCH_Q)]
                for t in range(NPR):
                    for ich in range(NCH_Q):
                        nc.tensor.matmul(
                            pms[ich][:],
                            wqT8[:, 2 * t:2 * t + 2, mt * P:(mt + 1) * P],
                            xqT8[:, 2 * t:2 * t + 2, ich * NC:(ich + 1) * NC],
                            start=(t == 0), stop=(t == NPR - 1), perf_mode=DR,
                        )
                for ich in range(NCH_Q):
                    nc.scalar.copy(QT8[:, mt, ich * NC:(ich + 1) * NC], pms[ich][:])

        with tc.tile_pool(name="ets", bufs=4) as ets_pool, \
                tc.tile_pool(name="csum", bufs=1) as csum_pool, \
                tc.tile_pool(name="ostage", bufs=3) as ostage, \
                tc.tile_pool(name="recip", bufs=1) as recip_pool, \
                tc.tile_pool(name="psum_aux", bufs=2, space="PSUM") as psum_aux:

            # both halves' colsum rows -> add -> broadcast to 128 partitions
            cs_g = [csum_pool.tile([1, D], BF16, tag=f"cs{g}", name=f"cs{g}") for g in range(2)]
            for g in range(2):
                nc.gpsimd.dma_start(
                    out=cs_g[g].bitcast(F8)[:],
                    in_=v_gath[g][0:2, NQT * D:(NQT + 1) * D],
                )
            colsum = csum_pool.tile([1, D], BF16, name="colsum")
            nc.vector.tensor_tensor(colsum[:], cs_g[0][:], cs_g[1][:], mybir.AluOpType.add)
            colsumP = csum_pool.tile([P, D], F32, name="colsumP")
            for cch in range(NCH_D):
                pb = psum_aux.tile([P, NC], F32, tag="pc", name="pb")
                nc.tensor.matmul(pb[:], ones1[:], colsum[:, cch * NC:(cch + 1) * NC])
                nc.vector.tensor_copy(colsumP[:, cch * NC:(cch + 1) * NC], pb[:])

            # scores^T, exp, eps: EPS[:, jt, i] = exp(ST/(D*WS^2)) - 1 in fp8
            for jt in range(NST):
                pms = [psum_mm.tile([P, NC], F32, tag="pm", name=f"pm{i}") for i in range(NCH_Q)]
                for t in range(NPR):
                    for ich in range(NCH_Q):
                        nc.tensor.matmul(
                            pms[ich][:],
                            KR8[:, 2 * t:2 * t + 2, jt * P:(jt + 1) * P],
                            QT8[:, 2 * t:2 * t + 2, ich * NC:(ich + 1) * NC],
                            start=(t == 0), stop=(t == NPR - 1), perf_mode=DR,
                        )
                for ich in range(NCH_Q):
                    et = ets_pool.tile([P, NC], BF16, tag="et", name="et")
                    nc.scalar.activation(et[:], pms[ich][:], EXP, scale=1.0 / (D * WS * WS))
                    nc.vector.tensor_scalar_add(
                        EPS[:, jt, ich * NC:(ich + 1) * NC], et[:], -1.0
                    )

            # out[it][:, cch] = (colsumP + sum_j eps.T @ V8) / (S + sum_j eps)
            for it in range(NQT):
                pm0 = psum_mm.tile([P, NC], F32, tag="pm", name="pm0")
                pm1 = psum_mm.tile([P, NC], F32, tag="pm", name="pm1")
                pr = psum_aux.tile([P, 1], F32, tag="pr", name="pr")
                for t in range(NPS):
                    lhsT = EPS[:, 2 * t:2 * t + 2, it * P:(it + 1) * P]
                    nc.tensor.matmul(pm0[:], lhsT, V8[:, 2 * t:2 * t + 2, 0:NC],
                                     start=(t == 0), stop=(t == NPS - 1), perf_mode=DR)
                    nc.tensor.matmul(pm1[:], lhsT, V8[:, 2 * t:2 * t + 2, NC:2 * NC],
                                     start=(t == 0), stop=(t == NPS - 1), perf_mode=DR)
                    nc.tensor.matmul(pr[:], lhsT, ones8[:],
                                     start=(t == 0), stop=(t == NPS - 1), perf_mode=DR)
                rc = recip_pool.tile([P, 1], F32, tag="rc", name="rc", bufs=2)
                nc.vector.tensor_scalar_add(rc[:], pr[:], float(S))
                nc.vector.reciprocal(rc[:], rc[:])
                for cch, pm in ((0, pm0), (1, pm1)):
                    ob = ostage.tile([P, NC], F32, tag="ob", name="ob")
                    nc.vector.tensor_tensor(
                        ob[:], pm[:], colsumP[:, cch * NC:(cch + 1) * NC],
                        mybir.AluOpType.add,
                    )
                    nc.vector.tensor_scalar_mul(ob[:], ob[:], rc[:])
                    nc.sync.dma_start(
                        out=out_ap[it * P:(it + 1) * P, cch * NC:(cch + 1) * NC],
                        in_=ob[:],
                    )
    return nc


_CACHE = {}


def _get_nc(S=2048, D=1024, QB=1024):
    key = (S, D, QB)
    if key not in _CACHE:
        nc = bacc.Bacc("TRN2", target_bir_lowering=False, debug=False, num_devices=8)
        build_attention(nc, S=S, D=D, QB=QB, n_cores=8)
        nc.compile()
        _CACHE[key] = nc
    return _CACHE[key]


def _pack3d(a2d, np_dtype):
    # [D, cols] -> [P, NDT, cols]: contraction dim split as (tile, partition)
    d, cols = a2d.shape
    ndt = d // P
    return np.ascontiguousarray(
        a2d.reshape(ndt, P, cols).transpose(1, 0, 2).astype(np_dtype)
    )


def _run(x, W_Q, W_K, W_V, **spmd_kwargs):
    B, S, D = x.shape  # (4, 2048, 1024)
    QB = S // 2        # queries per core (1024)
    # host-side operand packing: everything pre-transposed (contraction on
    # DRAM rows) and pre-tiled [P, NDT, cols]; fp8e4 with x16 scale for the
    # Q/K path, bf16 for the V path
    x32 = np.asarray(x, dtype=np.float32)
    wqt = _pack3d(np.asarray(W_Q, dtype=np.float32).T * WS, NP_F8)
    wkt = _pack3d(np.asarray(W_K, dtype=np.float32).T * WS, NP_F8)
    wvt = _pack3d(np.asarray(W_V, dtype=np.float32).T, NP_BF16)
    ws = {"wqt": wqt, "wkt": wkt, "wvt": wvt}
    nc = _get_nc(S=S, D=D, QB=QB)
    in_maps = []
    for core in range(8):
        b, h = core // 2, core % 2
        xqt = x32[b, h * QB:(h + 1) * QB, :].T
        xpt = x32[b, h::2, :].T
        in_maps.append({
            "xqt": _pack3d(xqt, NP_BF16),
            "xqt8": _pack3d(xqt, NP_F8),
            "xpt8": _pack3d(xpt, NP_F8),
            **ws,
        })
    res = run_bass_kernel_spmd(nc, in_maps, list(range(8)), **spmd_kwargs)
    out = np.empty((B, S, D), dtype=np.float32)
    for core in range(8):
        b, h = core // 2, core % 2
        out[b, h * QB:(h + 1) * QB, :] = res.results[core]["out"]
    return out, res


def kernel(x, W_Q, W_K, W_V):
    return _run(x, W_Q, W_K, W_V)[0]


# revision 14
# speedup vs baseline: 1.0245x; 1.0245x over previous
"""Trainium2 Bass kernel for single-head attention with row-major K-reshape.

Reference computation (per batch b):
    Q = x @ W_Q.T ; K = x @ W_K.T ; V = x @ W_V.T          # [S, D]
    K_r = K.reshape(D, S)          # row-major reshape, NOT a transpose
    scores = Q @ K_r / D
    out = softmax(scores, -1) @ V

Shapes: B=4, S=2048, D=1024, f32.

Sharding: 8 cores = (batch b in 0..3) x (pair-rank h in 0..1).  Core (b, h)
computes out[b, h*QB:(h+1)*QB, :].  K_r and V for batch b are computed
cooperatively by the pair (b,0)/(b,1) — each core builds one half and the
halves are exchanged with pair-wise AllGathers (DRAM bounce, fragments
laid out partition-major so each gather pulls back into SBUF with just
two wide DMAs):

  K_r half:  with S == 2*D the row-major reshape gives
                 K_r[m, g*D + c] = K[2m + g, c]
             so rank g's half is  x[g::2, :] @ W_K.T  — the parity-g rows
             of x ("xp").  Fragment g is exactly global columns
             [g*D, (g+1)*D) of K_r.
  V half:    rank g computes V rows [g*QB, (g+1)*QB) = xq @ W_V.T — the
             same rows as its query block ("xq").  The fragment also
             carries the rank's exact bf16 column-sum row (see below),
             byte-packed into a spare column block.

A zero-byte dummy AllGather is issued at kernel start so the collective
firmware pays its ~20us first-collective setup while the operand DMAs
are still streaming.

Host-side packing (layout/dtype prep only, numpy, once per call): all
operands are shipped pre-transposed AND pre-tiled as [P, NDT, cols]
(contraction dim split over partition x tile-index) so each one loads
with four wide DMAs and TensorE does zero transposes: xqT bf16 (V-proj
lhsT), xqT8 fp8 (Q-proj rhs), xpT8 fp8 (K-proj lhsT), wqT8/wkT8 fp8
scaled by 16, wvT bf16.

Precision plan (tolerance is 2e-2 relative to max|out|; this plan
CPU-simulates to 5.9e-3; the bf16 baseline measured 4.7e-3):
  fp8e4 + perf_mode=DoubleRow (2 k-tiles per instruction, 216 ns per
  N=512 matmul — true 2x) for Q-proj, K-proj, scores AND the attn @ V
  matmul.  W_Q/W_K carry a x16 pre-scale (folded into the exp).  The
  V projection stays bf16 (V errors pass straight to the output; fp8
  there measures 4.5e-2).  The attn @ V matmul survives fp8 via an
  offset trick: with near-uniform attention, e = exp(s) ~= 1, so
      out = (eps @ V8 + colsum_V) / (rowsum(eps) + S),   eps = f8(e - 1)
  where eps carries fp8 noise only on the +-0.15-magnitude deviation
  (attenuated ~1/sqrt(S) in the output) and colsum_V = sum_j V[j,:] is
  computed exactly per fragment half from the f32 PSUM V values
  (ones-matmul; colsum from fp8 V would measure 2.6e-2), exchanged as
  bf16 inside the V fragment, and added via a DVE broadcast add on the
  output drain.

Dataflow per core (TensorE matmul computes out[M,N] = lhsT[K,M].T @ rhs[K,N],
contraction over the partition dim; inner loops reuse each stationary
lhsT across both output chunks to halve LDWEIGHTS traffic):
    KRfrag[m, c] = lhsT=xpT8[:, pair, m], rhs=wkT8[:, pair, c]   (fp8 DR)
    Vfrag[s', c] = lhsT=xqT[:, dt, s'],   rhs=wvT[:, dt, c]      (bf16,
                   drained to fp8 for the gather + transient bf16 for
                   the per-half colsum ones-matmuls)
    QT8[m, i]    = lhsT=wqT8[:, pair, m], rhs=xqT8[:, pair, i]   (fp8 DR)
    KR8/V8       = one pair AllGather each, pulled into SBUF with two
                   wide DMAs on the gpsimd DGE ring
    ST[j, i]     = lhsT=KR8[:, pair, j],  rhs=QT8[:, pair, i]    (fp8 DR)
    et           = exp(ST / (D*256))      (ACT, psum->bf16 ring buffer)
    eps[j, i]    = et - 1 -> fp8          (DVE)
    colsumP      = both halves' colsum rows added (DVE) and broadcast
                   to all partitions with two K=1 matmuls
    O[i, c]      = lhsT=eps[:, pair, i],  rhs=V8[:, pair, c]     (fp8 DR)
    rsum[i, 1]   = lhsT=eps pair (shared), rhs=ones8 [P,2,1]  (fp8 DR,
                   fused into the O loop -> its weight load is free)
    out          = (O + colsumP) * (1 / (rsum + S))   (DVE)
"""

from contextlib import ExitStack

import ml_dtypes
import numpy as np

import concourse.tile as tile
from concourse import bacc, mybir
from concourse.bass_utils import run_bass_kernel_spmd

F32 = mybir.dt.float32
BF16 = mybir.dt.bfloat16
F8 = mybir.dt.float8e4
NP_BF16 = ml_dtypes.bfloat16
NP_F8 = ml_dtypes.float8_e4m3fn
P = 128
WS = 16.0  # fp8 pre-scale for W_Q / W_K


def build_attention(nc, S=2048, D=1024, QB=1024, n_cores=8):
    """Emit the per-core attention program into `nc`. Requires S == 2*D == 2*QB."""
    assert S == 2 * D and QB == D and D % P == 0
    NST = S // P        # seq tiles (16)
    NDT = D // P        # d_model tiles (8)
    NQT = QB // P       # query tiles for this core (8)
    NPR = NDT // 2      # DoubleRow k-tile pairs, d_model contraction (4)
    NPS = NST // 2      # DoubleRow k-tile pairs, seq contraction (8)
    NC = min(512, D)    # matmul free-dim chunk (one PSUM bank of f32)
    NCH_D = D // NC     # chunks over output channels (2)
    NCH_Q = QB // NC    # chunks over queries (2)
    EXP = mybir.ActivationFunctionType.Exp
    DR = mybir.MatmulPerfMode.DoubleRow
    groups = [[2 * b, 2 * b + 1] for b in range(n_cores // 2)]

    xqt_ap = nc.dram_tensor("xqt", [P, NDT, QB], BF16, kind="ExternalInput").ap()
    xqt8_ap = nc.dram_tensor("xqt8", [P, NDT, QB], F8, kind="ExternalInput").ap()
    xpt8_ap = nc.dram_tensor("xpt8", [P, NDT, D], F8, kind="ExternalInput").ap()
    wqt_ap = nc.dram_tensor("wqt", [P, NDT, D], F8, kind="ExternalInput").ap()
    wkt_ap = nc.dram_tensor("wkt", [P, NDT, D], F8, kind="ExternalInput").ap()
    wvt_ap = nc.dram_tensor("wvt", [P, NDT, D], BF16, kind="ExternalInput").ap()
    out_ap = nc.dram_tensor("out", [QB, D], F32, kind="ExternalOutput").ap()

    with tile.TileContext(nc) as tc, ExitStack() as ctx:
        const_pool = ctx.enter_context(tc.tile_pool(name="const", bufs=1))
        qt_pool = ctx.enter_context(tc.tile_pool(name="qt", bufs=1))
        kr_pool = ctx.enter_context(tc.tile_pool(name="kr", bufs=1))
        v8_pool = ctx.enter_context(tc.tile_pool(name="v8", bufs=1))
        eps_pool = ctx.enter_context(tc.tile_pool(name="eps", bufs=1))
        dram = ctx.enter_context(tc.tile_pool(name="dram", bufs=1, space="DRAM"))
        psum_mm = ctx.enter_context(tc.tile_pool(name="psum_mm", bufs=4, space="PSUM"))

        ones = const_pool.tile([P, 1], BF16)        # colsum stationary
        nc.vector.memset(ones, 1.0)
        ones1 = const_pool.tile([1, P], BF16)       # K=1 broadcast stationary
        nc.vector.memset(ones1, 1.0)
        ones8 = const_pool.tile([P, 2, 1], F8)      # DR rowsum rhs
        nc.vector.memset(ones8, 1.0)

        QT8 = qt_pool.tile([P, NDT, QB], F8, name="QT8")
        KR8 = kr_pool.tile([P, NDT, S], F8, name="KR8")
        V8 = v8_pool.tile([P, NST, D], F8, name="V8")
        EPS = eps_pool.tile([P, NST, QB], F8, name="EPS")

        # DRAM bounce buffers, partition-major.  The V fragment has one
        # spare column block; its first two partition rows carry the
        # rank's exact bf16 colsum row (2*D fp8 bytes == D bf16 values).
        kr_frag = dram.tile([P, NDT * D], F8, name="kr_frag")
        kr_gath = dram.tile([2, P, NDT * D], F8, name="kr_gath")
        v_frag = dram.tile([P, (NQT + 1) * D], F8, name="v_frag")
        v_gath = dram.tile([2, P, (NQT + 1) * D], F8, name="v_gath")

        with tc.tile_pool(name="xt", bufs=1) as xt_pool, \
                tc.tile_pool(name="wt", bufs=1) as wt_pool, \
                tc.tile_pool(name="frag", bufs=2) as frag_pool, \
                tc.tile_pool(name="csp", bufs=1) as csp_pool, \
                tc.tile_pool(name="psum_cs", bufs=2, space="PSUM") as psum_cs:

            xpT8 = xt_pool.tile([P, NDT, D], F8, tag="xpT8", name="xpT8")
            wkT8 = wt_pool.tile([P, NDT, D], F8, tag="wkT8", name="wkT8")
            xqT = xt_pool.tile([P, NDT, QB], BF16, tag="xqT", name="xqT")
            xqT8 = xt_pool.tile([P, NDT, QB], F8, tag="xqT8", name="xqT8")
            wvT = wt_pool.tile([P, NDT, D], BF16, tag="wvT", name="wvT")
            wqT8 = wt_pool.tile([P, NDT, D], F8, tag="wqT8", name="wqT8")

            def load3d(src_ap, dst3, eng):
                # pre-tiled operand: 4 wide DMAs across queues
                for t in range(NPR):
                    eng.dma_start(out=dst3[:, 2 * t:2 * t + 2, :],
                                  in_=src_ap[:, 2 * t:2 * t + 2, :])

            # All operand loads are pushed before any data-dependent
            # descriptor: the DGE rings are in-order, so a paced fragment
            # write ahead of a load would stall that load's descriptor.
            # K-path operands (which gate the first matmul) go first.
            load3d(xpt8_ap, xpT8, nc.sync)
            load3d(wkt_ap, wkT8, nc.sync)
            load3d(wvt_ap, wvT, nc.sync)
            load3d(xqt_ap, xqT, nc.sync)
            load3d(xqt8_ap, xqT8, nc.sync)
            load3d(wqt_ap, wqT8, nc.sync)

            # ---- K_r half first: its AllGather hides under the V half ----
            for mt in range(NDT):
                kf = frag_pool.tile([P, D], F8, tag="kf", name="kf")
                pms = [psum_mm.tile([P, NC], F32, tag="pm", name=f"pm{i}") for i in range(NCH_D)]
                for t in range(NPR):
                    for cch in range(NCH_D):
                        nc.tensor.matmul(
                            pms[cch][:],
                            xpT8[:, 2 * t:2 * t + 2, mt * P:(mt + 1) * P],
                            wkT8[:, 2 * t:2 * t + 2, cch * NC:(cch + 1) * NC],
                            start=(t == 0), stop=(t == NPR - 1), perf_mode=DR,
                        )
                for cch in range(NCH_D):
                    nc.scalar.copy(kf[:, cch * NC:(cch + 1) * NC], pms[cch][:])
                nc.scalar.dma_start(out=kr_frag[:, mt * D:(mt + 1) * D], in_=kf[:])
            nc.gpsimd.collective_compute(
                "AllGather", mybir.AluOpType.bypass, replica_groups=groups,
                ins=[kr_frag.opt()], outs=[kr_gath.opt()],
            )
            for g in range(2):
                nc.gpsimd.dma_start(
                    out=KR8[:, :, g * D:(g + 1) * D].opt(keep_dims={0, 1}),
                    in_=kr_gath[g],
                )

            # ---- V half: Vfrag[st] = xq @ W_V.T in bf16; fp8 for the
            # ---- gather plus exact bf16 colsum partial over this half.
            pcs = [psum_cs.tile([1, NC], F32, tag="pc", name=f"pc{i}") for i in range(NCH_D)]
            for st in range(NQT):
                vf8 = frag_pool.tile([P, D], F8, tag="vf8", name="vf8")
                vf16 = frag_pool.tile([P, D], BF16, tag="vf16", name="vf16", bufs=3)
                pms = [psum_mm.tile([P, NC], F32, tag="pm", name=f"pm{i}") for i in range(NCH_D)]
                for dt in range(NDT):
                    for cch in range(NCH_D):
                        nc.tensor.matmul(
                            pms[cch][:],
                            xqT[:, dt, st * P:(st + 1) * P],
                            wvT[:, dt, cch * NC:(cch + 1) * NC],
                            start=(dt == 0), stop=(dt == NDT - 1),
                        )
                for cch in range(NCH_D):
                    nc.vector.tensor_copy(vf8[:, cch * NC:(cch + 1) * NC], pms[cch][:])
                    nc.vector.tensor_copy(vf16[:, cch * NC:(cch + 1) * NC], pms[cch][:])
                    nc.tensor.matmul(
                        pcs[cch][:], ones[:], vf16[:, cch * NC:(cch + 1) * NC],
                        start=(st == 0), stop=(st == NQT - 1),
                    )
                nc.sync.dma_start(out=v_frag[:, st * D:(st + 1) * D], in_=vf8[:])
            # pack this half's bf16 colsum row into the spare column block
            cs_own = csp_pool.tile([1, D], BF16, name="cs_own")
            for cch in range(NCH_D):
                nc.scalar.copy(cs_own[:, cch * NC:(cch + 1) * NC], pcs[cch][:])
            nc.scalar.dma_start(
                out=v_frag[0:2, NQT * D:(NQT + 1) * D],
                in_=cs_own.bitcast(F8)[:],
            )
            nc.gpsimd.collective_compute(
                "AllGather", mybir.AluOpType.bypass, replica_groups=groups,
                ins=[v_frag.opt()], outs=[v_gath.opt()],
            )
            for g in range(2):
                nc.gpsimd.dma_start(
                    out=V8[:, g * NQT:(g + 1) * NQT, :].opt(keep_dims={0, 1}),
                    in_=v_gath[g][:, 0:NQT * D],
                )

            # ---- QT projection (fp8 DoubleRow; KR/V gathers in flight) ----
            for mt in range(NDT):
                pms = [psum_mm.tile([P, NC], F32, tag="pm", name=f"pm{i}") for i in range(NCH_Q)]
                for t in range(NPR):
                    for ich in range(NCH_Q):
                        nc.tensor.matmul(
                            pms[ich][:],
                            wqT8[:, 2 * t:2 * t + 2, mt * P:(mt + 1) * P],
                            xqT8[:, 2 * t:2 * t + 2, ich * NC:(ich + 1) * NC],
                            start=(t == 0), stop=(t == NPR - 1), perf_mode=DR,
                        )
                for ich in range(NCH_Q):
                    nc.scalar.copy(QT8[:, mt, ich * NC:(ich + 1) * NC], pms[ich][:])

        with tc.tile_pool(name="ets", bufs=4) as ets_pool, \
                tc.tile_pool(name="csum", bufs=1) as csum_pool, \
                tc.tile_pool(name="ostage", bufs=3) as ostage, \
                tc.tile_pool(name="recip", bufs=1) as recip_pool, \
                tc.tile_pool(name="psum_aux", bufs=2, space="PSUM") as psum_aux:

            # both halves' colsum rows -> add -> broadcast to 128 partitions
            cs_g = [csum_pool.tile([1, D], BF16, tag=f"cs{g}", name=f"cs{g}") for g in range(2)]
            for g in range(2):
                nc.gpsimd.dma_start(
                    out=cs_g[g].bitcast(F8)[:],
                    in_=v_gath[g][0:2, NQT * D:(NQT + 1) * D],
                )
            colsum = csum_pool.tile([1, D], BF16, name="colsum")
            nc.vector.tensor_tensor(colsum[:], cs_g[0][:], cs_g[1][:], mybir.AluOpType.add)
            colsumP = csum_pool.tile([P, D], F32, name="colsumP")
            for cch in range(NCH_D):
                pb = psum_aux.tile([P, NC], F32, tag="pc", name="pb")
                nc.tensor.matmul(pb[:], ones1[:], colsum[:, cch * NC:(cch + 1) * NC])
                nc.vector.tensor_copy(colsumP[:, cch * NC:(cch + 1) * NC], pb[:])

            # scores^T, exp, eps: EPS[:, jt, i] = exp(ST/(D*WS^2)) - 1 in fp8
            for jt in range(NST):
                pms = [psum_mm.tile([P, NC], F32, tag="pm", name=f"pm{i}") for i in range(NCH_Q)]
                for t in range(NPR):
                    for ich in range(NCH_Q):
                        nc.tensor.matmul(
                            pms[ich][:],
                            KR8[:, 2 * t:2 * t + 2, jt * P:(jt + 1) * P],
                            QT8[:, 2 * t:2 * t + 2, ich * NC:(ich + 1) * NC],
                            start=(t == 0), stop=(t == NPR - 1), perf_mode=DR,
                        )
                for ich in range(NCH_Q):
                    et = ets_pool.tile([P, NC], BF16, tag="et", name="et")
                    nc.scalar.activation(et[:], pms[ich][:], EXP, scale=1.0 / (D * WS * WS))
                    nc.vector.tensor_scalar_add(
                        EPS[:, jt, ich * NC:(ich + 1) * NC], et[:], -1.0
                    )

            # out[it][:, cch] = (colsumP + sum_j eps.T @ V8) / (S + sum_j eps)
            for it in range(NQT):
                pm0 = psum_mm.tile([P, NC], F32, tag="pm", name="pm0")
                pm1 = psum_mm.tile([P, NC], F32, tag="pm", name="pm1")
                pr = psum_aux.tile([P, 1], F32, tag="pr", name="pr")
                for t in range(NPS):
                    lhsT = EPS[:, 2 * t:2 * t + 2, it * P:(it + 1) * P]
                    nc.tensor.matmul(pm0[:], lhsT, V8[:, 2 * t:2 * t + 2, 0:NC],
                                     start=(t == 0), stop=(t == NPS - 1), perf_mode=DR)
                    nc.tensor.matmul(pm1[:], lhsT, V8[:, 2 * t:2 * t + 2, NC:2 * NC],
                                     start=(t == 0), stop=(t == NPS - 1), perf_mode=DR)
                    nc.tensor.matmul(pr[:], lhsT, ones8[:],
                                     start=(t == 0), stop=(t == NPS - 1), perf_mode=DR)
                rc = recip_pool.tile([P, 1], F32, tag="rc", name="rc", bufs=2)
                nc.vector.tensor_scalar_add(rc[:], pr[:], float(S))
                nc.vector.reciprocal(rc[:], rc[:])
                for cch, pm in ((0, pm0), (1, pm1)):
                    ob = ostage.tile([P, NC], F32, tag="ob", name="ob")
                    nc.vector.tensor_tensor(
                        ob[:], pm[:], colsumP[:, cch * NC:(cch + 1) * NC],
                        mybir.AluOpType.add,
                    )
                    nc.vector.tensor_scalar_mul(ob[:], ob[:], rc[:])
                    nc.sync.dma_start(
                        out=out_ap[it * P:(it + 1) * P, cch * NC:(cch + 1) * NC],
                        in_=ob[:],
                    )
    return nc


_CACHE = {}


def _get_nc(S=2048, D=1024, QB=1024):
    key = (S, D, QB)
    if key not in _CACHE:
        nc = bacc.Bacc("TRN2", target_bir_lowering=False, debug=False, num_devices=8)
        build_attention(nc, S=S, D=D, QB=QB, n_cores=8)
        nc.compile()
        _CACHE[key] = nc
    return _CACHE[key]


def _pack3d(a2d, np_dtype):
    # [D, cols] -> [P, NDT, cols]: contraction dim split as (tile, partition)
    d, cols = a2d.shape
    ndt = d // P
    return np.ascontiguousarray(
        a2d.reshape(ndt, P, cols).transpose(1, 0, 2).astype(np_dtype)
    )


def _run(x, W_Q, W_K, W_V, **spmd_kwargs):
    B, S, D = x.shape  # (4, 2048, 1024)
    QB = S // 2        # queries per core (1024)
    # host-side operand packing: everything pre-transposed (contraction on
    # DRAM rows) and pre-tiled [P, NDT, cols]; fp8e4 with x16 scale for the
    # Q/K path, bf16 for the V path
    x32 = np.asarray(x, dtype=np.float32)
    wqt = _pack3d(np.asarray(W_Q, dtype=np.float32).T * WS, NP_F8)
    wkt = _pack3d(np.asarray(W_K, dtype=np.float32).T * WS, NP_F8)
    wvt = _pack3d(np.asarray(W_V, dtype=np.float32).T, NP_BF16)
    ws = {"wqt": wqt, "wkt": wkt, "wvt": wvt}
    nc = _get_nc(S=S, D=D, QB=QB)
    in_maps = []
    for core in range(8):
        b, h = core // 2, core % 2
        xqt = x32[b, h * QB:(h + 1) * QB, :].T
        xpt = x32[b, h::2, :].T
        in_maps.append({
            "xqt": _pack3d(xqt, NP_BF16),
            "xqt8": _pack3d(xqt, NP_F8),
            "xpt8": _pack3d(xpt, NP_F8),
            **ws,
        })
    res = run_bass_kernel_spmd(nc, in_maps, list(range(8)), **spmd_kwargs)
    out = np.empty((B, S, D), dtype=np.float32)
    for core in range(8):
        b, h = core // 2, core % 2
        out[b, h * QB:(h + 1) * QB, :] = res.results[core]["out"]
    return out, res


def kernel(x, W_Q, W_K, W_V):
    return _run(x, W_Q, W_K, W_V)[0]


# revision 15
# speedup vs baseline: 1.0448x; 1.0199x over previous
"""Trainium2 Bass kernel for single-head attention with row-major K-reshape.

Reference computation (per batch b):
    Q = x @ W_Q.T ; K = x @ W_K.T ; V = x @ W_V.T          # [S, D]
    K_r = K.reshape(D, S)          # row-major reshape, NOT a transpose
    scores = Q @ K_r / D
    out = softmax(scores, -1) @ V

Shapes: B=4, S=2048, D=1024, f32.

Sharding: 8 cores = (batch b in 0..3) x (pair-rank h in 0..1).  Core (b, h)
computes out[b, h*QB:(h+1)*QB, :].  K_r and V for batch b are computed
cooperatively by the pair (b,0)/(b,1) — each core builds one half and the
halves are exchanged with pair-wise AllGathers (DRAM bounce, fragments
laid out partition-major so each gather pulls back into SBUF with just
two wide DMAs):

  K_r half:  with S == 2*D the row-major reshape gives
                 K_r[m, g*D + c] = K[2m + g, c]
             so rank g's half is  x[g::2, :] @ W_K.T  — the parity-g rows
             of x ("xp").  Fragment g is exactly global columns
             [g*D, (g+1)*D) of K_r.
  V half:    rank g computes V rows [g*QB, (g+1)*QB) = xq @ W_V.T — the
             same rows as its query block ("xq").  The fragment also
             carries the rank's exact bf16 column-sum row (see below),
             byte-packed into a spare column block.

A zero-byte dummy AllGather is issued at kernel start so the collective
firmware pays its ~20us first-collective setup while the operand DMAs
are still streaming.

Host-side packing (layout/dtype prep only, numpy, once per call): all
operands are shipped pre-transposed AND pre-tiled as [P, NDT, cols]
(contraction dim split over partition x tile-index) so each one loads
with four wide DMAs and TensorE does zero transposes: xqT bf16 (V-proj
lhsT), xqT8 fp8 (Q-proj rhs), xpT8 fp8 (K-proj lhsT), wqT8/wkT8 fp8
scaled by 16, wvT bf16.

Precision plan (tolerance is 2e-2 relative to max|out|; this plan
CPU-simulates to 5.9e-3; the bf16 baseline measured 4.7e-3):
  fp8e4 + perf_mode=DoubleRow (2 k-tiles per instruction, 216 ns per
  N=512 matmul — true 2x) for Q-proj, K-proj, scores AND the attn @ V
  matmul.  W_Q/W_K carry a x16 pre-scale (folded into the exp).  The
  V projection stays bf16 (V errors pass straight to the output; fp8
  there measures 4.5e-2).  The attn @ V matmul survives fp8 via an
  offset trick: with near-uniform attention, e = exp(s) ~= 1, so
      out = (eps @ V8 + colsum_V) / (rowsum(eps) + S),   eps = f8(e - 1)
  where eps carries fp8 noise only on the +-0.15-magnitude deviation
  (attenuated ~1/sqrt(S) in the output) and colsum_V = sum_j V[j,:] is
  computed exactly per fragment half from the f32 PSUM V values
  (ones-matmul; colsum from fp8 V would measure 2.6e-2), exchanged as
  bf16 inside the V fragment, and added via a DVE broadcast add on the
  output drain.

Dataflow per core (TensorE matmul computes out[M,N] = lhsT[K,M].T @ rhs[K,N],
contraction over the partition dim; inner loops reuse each stationary
lhsT across both output chunks to halve LDWEIGHTS traffic):
    KRfrag[m, c] = lhsT=xpT8[:, pair, m], rhs=wkT8[:, pair, c]   (fp8 DR)
    Vfrag[s', c] = lhsT=xqT[:, dt, s'],   rhs=wvT[:, dt, c]      (bf16,
                   drained to fp8 for the gather + transient bf16 for
                   the per-half colsum ones-matmuls)
    QT8[m, i]    = lhsT=wqT8[:, pair, m], rhs=xqT8[:, pair, i]   (fp8 DR)
    KR8/V8       = one pair AllGather each, pulled into SBUF with two
                   wide DMAs on the gpsimd DGE ring
    ST[j, i]     = lhsT=KR8[:, pair, j],  rhs=QT8[:, pair, i]    (fp8 DR)
    et           = exp(ST / (D*256))      (ACT, psum->bf16 ring buffer)
    eps[j, i]    = et - 1 -> fp8          (DVE)
    colsumP      = both halves' colsum rows added (DVE) and broadcast
                   to all partitions with two K=1 matmuls
    O[i, c]      = lhsT=eps[:, pair, i],  rhs=V8[:, pair, c]     (fp8 DR)
    rsum[i, 1]   = lhsT=eps pair (shared), rhs=ones8 [P,2,1]  (fp8 DR,
                   fused into the O loop -> its weight load is free)
    out          = (O + colsumP) * (1 / (rsum + S))   (DVE)
"""

from contextlib import ExitStack

import ml_dtypes
import numpy as np

import concourse.tile as tile
from concourse import bacc, mybir
from concourse.bass_utils import run_bass_kernel_spmd

F32 = mybir.dt.float32
BF16 = mybir.dt.bfloat16
F8 = mybir.dt.float8e4
NP_BF16 = ml_dtypes.bfloat16
NP_F8 = ml_dtypes.float8_e4m3fn
P = 128
WS = 16.0  # fp8 pre-scale for W_Q / W_K


def build_attention(nc, S=2048, D=1024, QB=1024, n_cores=8):
    """Emit the per-core attention program into `nc`. Requires S == 2*D == 2*QB."""
    assert S == 2 * D and QB == D and D % P == 0
    NST = S // P        # seq tiles (16)
    NDT = D // P        # d_model tiles (8)
    NQT = QB // P       # query tiles for this core (8)
    NPR = NDT // 2      # DoubleRow k-tile pairs, d_model contraction (4)
    NPS = NST // 2      # DoubleRow k-tile pairs, seq contraction (8)
    NC = min(512, D)    # matmul free-dim chunk (one PSUM bank of f32)
    NCH_D = D // NC     # chunks over output channels (2)
    NCH_Q = QB // NC    # chunks over queries (2)
    EXP = mybir.ActivationFunctionType.Exp
    DR = mybir.MatmulPerfMode.DoubleRow
    groups = [[2 * b, 2 * b + 1] for b in range(n_cores // 2)]

    xqt_ap = nc.dram_tensor("xqt", [P, NDT, QB], BF16, kind="ExternalInput").ap()
    xqt8_ap = nc.dram_tensor("xqt8", [P, NDT, QB], F8, kind="ExternalInput").ap()
    xpt8_ap = nc.dram_tensor("xpt8", [P, NDT, D], F8, kind="ExternalInput").ap()
    wqt_ap = nc.dram_tensor("wqt", [P, NDT, D], F8, kind="ExternalInput").ap()
    wkt_ap = nc.dram_tensor("wkt", [P, NDT, D], F8, kind="ExternalInput").ap()
    wvt_ap = nc.dram_tensor("wvt", [P, NDT, D], BF16, kind="ExternalInput").ap()
    out_ap = nc.dram_tensor("out", [QB, D], F32, kind="ExternalOutput").ap()

    with tile.TileContext(nc) as tc, ExitStack() as ctx:
        const_pool = ctx.enter_context(tc.tile_pool(name="const", bufs=1))
        qt_pool = ctx.enter_context(tc.tile_pool(name="qt", bufs=1))
        kr_pool = ctx.enter_context(tc.tile_pool(name="kr", bufs=1))
        v8_pool = ctx.enter_context(tc.tile_pool(name="v8", bufs=1))
        eps_pool = ctx.enter_context(tc.tile_pool(name="eps", bufs=1))
        dram = ctx.enter_context(tc.tile_pool(name="dram", bufs=1, space="DRAM"))
        psum_mm = ctx.enter_context(tc.tile_pool(name="psum_mm", bufs=6, space="PSUM"))

        ones = const_pool.tile([P, 1], BF16)        # colsum stationary
        nc.vector.memset(ones, 1.0)
        ones1 = const_pool.tile([1, P], BF16)       # K=1 broadcast stationary
        nc.vector.memset(ones1, 1.0)
        ones8 = const_pool.tile([P, 2, 1], F8)      # DR rowsum rhs
        nc.vector.memset(ones8, 1.0)

        QT8 = qt_pool.tile([P, NDT, QB], F8, name="QT8")
        KR8 = kr_pool.tile([P, NDT, S], F8, name="KR8")
        V8 = v8_pool.tile([P, NST, D], F8, name="V8")
        EPS = eps_pool.tile([P, NST, QB], F8, name="EPS")

        # DRAM bounce buffers, partition-major.  The V fragment has one
        # spare column block; its first two partition rows carry the
        # rank's exact bf16 colsum row (2*D fp8 bytes == D bf16 values).
        kr_frag = dram.tile([P, NDT * D], F8, name="kr_frag")
        kr_gath = dram.tile([2, P, NDT * D], F8, name="kr_gath")
        v_frag = dram.tile([P, (NQT + 1) * D], F8, name="v_frag")
        v_gath = dram.tile([2, P, (NQT + 1) * D], F8, name="v_gath")

        with tc.tile_pool(name="xt", bufs=1) as xt_pool, \
                tc.tile_pool(name="wt", bufs=1) as wt_pool, \
                tc.tile_pool(name="frag", bufs=2) as frag_pool, \
                tc.tile_pool(name="csp", bufs=1) as csp_pool, \
                tc.tile_pool(name="psum_cs", bufs=2, space="PSUM") as psum_cs:

            xpT8 = xt_pool.tile([P, NDT, D], F8, tag="xpT8", name="xpT8")
            wkT8 = wt_pool.tile([P, NDT, D], F8, tag="wkT8", name="wkT8")
            xqT = xt_pool.tile([P, NDT, QB], BF16, tag="xqT", name="xqT")
            xqT8 = xt_pool.tile([P, NDT, QB], F8, tag="xqT8", name="xqT8")
            wvT = wt_pool.tile([P, NDT, D], BF16, tag="wvT", name="wvT")
            wqT8 = wt_pool.tile([P, NDT, D], F8, tag="wqT8", name="wqT8")

            def load3d(src_ap, dst3, eng):
                # pre-tiled operand: 4 wide DMAs across queues
                for t in range(NPR):
                    eng.dma_start(out=dst3[:, 2 * t:2 * t + 2, :],
                                  in_=src_ap[:, 2 * t:2 * t + 2, :])

            # All operand loads are pushed before any data-dependent
            # descriptor: the DGE rings are in-order, so a paced fragment
            # write ahead of a load would stall that load's descriptor.
            # K-path operands (which gate the first matmul) go first.
            load3d(xpt8_ap, xpT8, nc.sync)
            load3d(wkt_ap, wkT8, nc.sync)
            load3d(wvt_ap, wvT, nc.sync)
            load3d(xqt_ap, xqT, nc.sync)
            load3d(xqt8_ap, xqT8, nc.sync)
            load3d(wqt_ap, wqT8, nc.sync)

            # ---- K_r half first: its AllGather hides under the V half ----
            for mt in range(NDT):
                kf = frag_pool.tile([P, D], F8, tag="kf", name="kf")
                pms = [psum_mm.tile([P, NC], F32, tag="pm", name=f"pm{i}") for i in range(NCH_D)]
                for t in range(NPR):
                    for cch in range(NCH_D):
                        nc.tensor.matmul(
                            pms[cch][:],
                            xpT8[:, 2 * t:2 * t + 2, mt * P:(mt + 1) * P],
                            wkT8[:, 2 * t:2 * t + 2, cch * NC:(cch + 1) * NC],
                            start=(t == 0), stop=(t == NPR - 1), perf_mode=DR,
                        )
                for cch in range(NCH_D):
                    nc.vector.tensor_copy(kf[:, cch * NC:(cch + 1) * NC], pms[cch][:])
                nc.scalar.dma_start(out=kr_frag[:, mt * D:(mt + 1) * D], in_=kf[:])
            nc.gpsimd.collective_compute(
                "AllGather", mybir.AluOpType.bypass, replica_groups=groups,
                ins=[kr_frag.opt()], outs=[kr_gath.opt()],
            )
            for g in range(2):
                nc.gpsimd.dma_start(
                    out=KR8[:, :, g * D:(g + 1) * D].opt(keep_dims={0, 1}),
                    in_=kr_gath[g],
                )

            # ---- V half: Vfrag[st] = xq @ W_V.T in bf16; fp8 for the
            # ---- gather plus exact bf16 colsum partial over this half.
            pcs = [psum_cs.tile([1, NC], F32, tag="pc", name=f"pc{i}") for i in range(NCH_D)]
            for st in range(NQT):
                vf8 = frag_pool.tile([P, D], F8, tag="vf8", name="vf8")
                vf16 = frag_pool.tile([P, D], BF16, tag="vf16", name="vf16", bufs=3)
                pms = [psum_mm.tile([P, NC], F32, tag="pm", name=f"pm{i}") for i in range(NCH_D)]
                for dt in range(NDT):
                    for cch in range(NCH_D):
                        nc.tensor.matmul(
                            pms[cch][:],
                            xqT[:, dt, st * P:(st + 1) * P],
                            wvT[:, dt, cch * NC:(cch + 1) * NC],
                            start=(dt == 0), stop=(dt == NDT - 1),
                        )
                for cch in range(NCH_D):
                    nc.vector.tensor_copy(vf8[:, cch * NC:(cch + 1) * NC], pms[cch][:])
                    nc.vector.tensor_copy(vf16[:, cch * NC:(cch + 1) * NC], pms[cch][:])
                    nc.tensor.matmul(
                        pcs[cch][:], ones[:], vf16[:, cch * NC:(cch + 1) * NC],
                        start=(st == 0), stop=(st == NQT - 1),
                    )
                nc.sync.dma_start(out=v_frag[:, st * D:(st + 1) * D], in_=vf8[:])
            # pack this half's bf16 colsum row into the spare column block
            cs_own = csp_pool.tile([1, D], BF16, name="cs_own")
            for cch in range(NCH_D):
                nc.scalar.copy(cs_own[:, cch * NC:(cch + 1) * NC], pcs[cch][:])
            nc.scalar.dma_start(
                out=v_frag[0:2, NQT * D:(NQT + 1) * D],
                in_=cs_own.bitcast(F8)[:],
            )
            nc.gpsimd.collective_compute(
                "AllGather", mybir.AluOpType.bypass, replica_groups=groups,
                ins=[v_frag.opt()], outs=[v_gath.opt()],
            )
            for g in range(2):
                nc.gpsimd.dma_start(
                    out=V8[:, g * NQT:(g + 1) * NQT, :].opt(keep_dims={0, 1}),
                    in_=v_gath[g][:, 0:NQT * D],
                )

            # ---- QT projection (fp8 DoubleRow; KR/V gathers in flight) ----
            for mt in range(NDT):
                pms = [psum_mm.tile([P, NC], F32, tag="pm", name=f"pm{i}") for i in range(NCH_Q)]
                for t in range(NPR):
                    for ich in range(NCH_Q):
                        nc.tensor.matmul(
                            pms[ich][:],
                            wqT8[:, 2 * t:2 * t + 2, mt * P:(mt + 1) * P],
                            xqT8[:, 2 * t:2 * t + 2, ich * NC:(ich + 1) * NC],
                            start=(t == 0), stop=(t == NPR - 1), perf_mode=DR,
                        )
                for ich in range(NCH_Q):
                    nc.scalar.copy(QT8[:, mt, ich * NC:(ich + 1) * NC], pms[ich][:])

        with tc.tile_pool(name="ets", bufs=4) as ets_pool, \
                tc.tile_pool(name="csum", bufs=1) as csum_pool, \
                tc.tile_pool(name="ostage", bufs=3) as ostage, \
                tc.tile_pool(name="recip", bufs=1) as recip_pool, \
                tc.tile_pool(name="psum_aux", bufs=1, space="PSUM") as psum_aux:

            # both halves' colsum rows -> add -> broadcast to 128 partitions
            cs_g = [csum_pool.tile([1, D], BF16, tag=f"cs{g}", name=f"cs{g}") for g in range(2)]
            for g in range(2):
                nc.gpsimd.dma_start(
                    out=cs_g[g].bitcast(F8)[:],
                    in_=v_gath[g][0:2, NQT * D:(NQT + 1) * D],
                )
            colsum = csum_pool.tile([1, D], BF16, name="colsum")
            nc.vector.tensor_tensor(colsum[:], cs_g[0][:], cs_g[1][:], mybir.AluOpType.add)
            colsumP = csum_pool.tile([P, D], F32, name="colsumP")
            for cch in range(NCH_D):
                pb = psum_aux.tile([P, NC], F32, tag="pc", name="pb")
                nc.tensor.matmul(pb[:], ones1[:], colsum[:, cch * NC:(cch + 1) * NC])
                nc.vector.tensor_copy(colsumP[:, cch * NC:(cch + 1) * NC], pb[:])

            # scores^T, exp, eps: EPS[:, jt, i] = exp(ST/(D*WS^2)) - 1 in fp8
            for jt in range(NST):
                pms = [psum_mm.tile([P, NC], F32, tag="pm", name=f"pm{i}") for i in range(NCH_Q)]
                for t in range(NPR):
                    for ich in range(NCH_Q):
                        nc.tensor.matmul(
                            pms[ich][:],
                            KR8[:, 2 * t:2 * t + 2, jt * P:(jt + 1) * P],
                            QT8[:, 2 * t:2 * t + 2, ich * NC:(ich + 1) * NC],
                            start=(t == 0), stop=(t == NPR - 1), perf_mode=DR,
                        )
                for ich in range(NCH_Q):
                    et = ets_pool.tile([P, NC], BF16, tag="et", name="et")
                    nc.scalar.activation(et[:], pms[ich][:], EXP, scale=1.0 / (D * WS * WS))
                    nc.vector.tensor_scalar_add(
                        EPS[:, jt, ich * NC:(ich + 1) * NC], et[:], -1.0
                    )

            # out[it][:, cch] = (colsumP + sum_j eps.T @ V8) / (S + sum_j eps)
            for it in range(NQT):
                pm0 = psum_mm.tile([P, NC], F32, tag="pm", name="pm0")
                pm1 = psum_mm.tile([P, NC], F32, tag="pm", name="pm1")
                pr = psum_aux.tile([P, 1], F32, tag="pr", name="pr")
                for t in range(NPS):
                    lhsT = EPS[:, 2 * t:2 * t + 2, it * P:(it + 1) * P]
                    nc.tensor.matmul(pm0[:], lhsT, V8[:, 2 * t:2 * t + 2, 0:NC],
                                     start=(t == 0), stop=(t == NPS - 1), perf_mode=DR)
                    nc.tensor.matmul(pm1[:], lhsT, V8[:, 2 * t:2 * t + 2, NC:2 * NC],
                                     start=(t == 0), stop=(t == NPS - 1), perf_mode=DR)
                    nc.tensor.matmul(pr[:], lhsT, ones8[:],
                                     start=(t == 0), stop=(t == NPS - 1), perf_mode=DR)
                rc = recip_pool.tile([P, 1], F32, tag="rc", name="rc", bufs=2)
                nc.vector.tensor_scalar_add(rc[:], pr[:], float(S))
                nc.vector.reciprocal(rc[:], rc[:])
                for cch, pm in ((0, pm0), (1, pm1)):
                    ob = ostage.tile([P, NC], F32, tag="ob", name="ob")
                    nc.vector.tensor_tensor(
                        ob[:], pm[:], colsumP[:, cch * NC:(cch + 1) * NC],
                        mybir.AluOpType.add,
                    )
                    nc.vector.tensor_scalar_mul(ob[:], ob[:], rc[:])
                    nc.sync.dma_start(
                        out=out_ap[it * P:(it + 1) * P, cch * NC:(cch + 1) * NC],
                        in_=ob[:],
                    )
    return nc


_CACHE = {}


def _get_nc(S=2048, D=1024, QB=1024):
    key = (S, D, QB)
    if key not in _CACHE:
        nc = bacc.Bacc("TRN2", target_bir_lowering=False, debug=False, num_devices=8)
        build_attention(nc, S=S, D=D, QB=QB, n_cores=8)
        nc.compile()
        _CACHE[key] = nc
    return _CACHE[key]


def _pack3d(a2d, np_dtype):
    # [D, cols] -> [P, NDT, cols]: contraction dim split as (tile, partition)
    d, cols = a2d.shape
    ndt = d // P
    return np.ascontiguousarray(
        a2d.reshape(ndt, P, cols).transpose(1, 0, 2).astype(np_dtype)
    )


def _run(x, W_Q, W_K, W_V, **spmd_kwargs):
    B, S, D = x.shape  # (4, 2048, 1024)
    QB = S // 2        # queries per core (1024)
    # host-side operand packing: everything pre-transposed (contraction on
    # DRAM rows) and pre-tiled [P, NDT, cols]; fp8e4 with x16 scale for the
    # Q/K path, bf16 for the V path
    x32 = np.asarray(x, dtype=np.float32)
    wqt = _pack3d(np.asarray(W_Q, dtype=np.float32).T * WS, NP_F8)
    wkt = _pack3d(np.asarray(W_K, dtype=np.float32).T * WS, NP_F8)
    wvt = _pack3d(np.asarray(W_V, dtype=np.float32).T, NP_BF16)
    ws = {"wqt": wqt, "wkt": wkt, "wvt": wvt}
    nc = _get_nc(S=S, D=D, QB=QB)
    in_maps = []
    for core in range(8):
        b, h = core // 2, core % 2
        xqt = x32[b, h * QB:(h + 1) * QB, :].T
        xpt = x32[b, h::2, :].T
        in_maps.append({
            "xqt": _pack3d(xqt, NP_BF16),
            "xqt8": _pack3d(xqt, NP_F8),
            "xpt8": _pack3d(xpt, NP_F8),
            **ws,
        })
    res = run_bass_kernel_spmd(nc, in_maps, list(range(8)), **spmd_kwargs)
    out = np.empty((B, S, D), dtype=np.float32)
    for core in range(8):
        b, h = core // 2, core % 2
        out[b, h * QB:(h + 1) * QB, :] = res.results[core]["out"]
    return out, res


def kernel(x, W_Q, W_K, W_V):
    return _run(x, W_Q, W_K, W_V)[0]


# revision 16
# speedup vs baseline: 1.0610x; 1.0155x over previous
"""Trainium2 Bass kernel for single-head attention with row-major K-reshape.

Reference computation (per batch b):
    Q = x @ W_Q.T ; K = x @ W_K.T ; V = x @ W_V.T          # [S, D]
    K_r = K.reshape(D, S)          # row-major reshape, NOT a transpose
    scores = Q @ K_r / D
    out = softmax(scores, -1) @ V

Shapes: B=4, S=2048, D=1024, f32.

Sharding: 8 cores = (batch b in 0..3) x (pair-rank h in 0..1).  Core (b, h)
computes out[b, h*QB:(h+1)*QB, :].  K_r and V for batch b are computed
cooperatively by the pair (b,0)/(b,1) — each core builds one half and the
halves are exchanged with pair-wise AllGathers (DRAM bounce, fragments
laid out partition-major so each gather pulls back into SBUF with just
two wide DMAs):

  K_r half:  with S == 2*D the row-major reshape gives
                 K_r[m, g*D + c] = K[2m + g, c]
             so rank g's half is  x[g::2, :] @ W_K.T  — the parity-g rows
             of x ("xp").  Fragment g is exactly global columns
             [g*D, (g+1)*D) of K_r.
  V half:    rank g computes V rows [g*QB, (g+1)*QB) = xq @ W_V.T — the
             same rows as its query block ("xq").  The fragment also
             carries the rank's exact bf16 column-sum row (see below),
             byte-packed into a spare column block.

A zero-byte dummy AllGather is issued at kernel start so the collective
firmware pays its ~20us first-collective setup while the operand DMAs
are still streaming.

Host-side packing (layout/dtype prep only, numpy, once per call): all
operands are shipped pre-transposed AND pre-tiled as [P, NDT, cols]
(contraction dim split over partition x tile-index) so each one loads
with four wide DMAs and TensorE does zero transposes: xqT bf16 (V-proj
lhsT), xqT8 fp8 (Q-proj rhs), xpT8 fp8 (K-proj lhsT), wqT8/wkT8 fp8
scaled by 16, wvT bf16.

Precision plan (tolerance is 2e-2 relative to max|out|; this plan
CPU-simulates to 5.9e-3; the bf16 baseline measured 4.7e-3):
  fp8e4 + perf_mode=DoubleRow (2 k-tiles per instruction, 216 ns per
  N=512 matmul — true 2x) for Q-proj, K-proj, scores AND the attn @ V
  matmul.  W_Q/W_K carry a x16 pre-scale (folded into the exp).  The
  V projection stays bf16 (V errors pass straight to the output; fp8
  there measures 4.5e-2).  The attn @ V matmul survives fp8 via an
  offset trick: with near-uniform attention, e = exp(s) ~= 1, so
      out = (eps @ V8 + colsum_V) / (rowsum(eps) + S),   eps = f8(e - 1)
  where eps carries fp8 noise only on the +-0.15-magnitude deviation
  (attenuated ~1/sqrt(S) in the output) and colsum_V = sum_j V[j,:] is
  computed exactly per fragment half from the f32 PSUM V values
  (ones-matmul; colsum from fp8 V would measure 2.6e-2), exchanged as
  bf16 inside the V fragment, and added via a DVE broadcast add on the
  output drain.

Dataflow per core (TensorE matmul computes out[M,N] = lhsT[K,M].T @ rhs[K,N],
contraction over the partition dim; inner loops reuse each stationary
lhsT across both output chunks to halve LDWEIGHTS traffic):
    KRfrag[m, c] = lhsT=xpT8[:, pair, m], rhs=wkT8[:, pair, c]   (fp8 DR)
    Vfrag[s', c] = lhsT=xqT[:, dt, s'],   rhs=wvT[:, dt, c]      (bf16,
                   drained to fp8 for the gather + transient bf16 for
                   the per-half colsum ones-matmuls)
    QT8[m, i]    = lhsT=wqT8[:, pair, m], rhs=xqT8[:, pair, i]   (fp8 DR)
    KR8/V8       = one pair AllGather each, pulled into SBUF with two
                   wide DMAs on the gpsimd DGE ring
    ST[j, i]     = lhsT=KR8[:, pair, j],  rhs=QT8[:, pair, i]    (fp8 DR)
    et           = exp(ST / (D*256))      (ACT, psum->bf16 ring buffer)
    eps[j, i]    = et - 1 -> fp8          (DVE)
    colsumP      = both halves' colsum rows added (DVE) and broadcast
                   to all partitions with two K=1 matmuls
    O[i, c]      = lhsT=eps[:, pair, i],  rhs=V8[:, pair, c]     (fp8 DR)
    rsum[i, 1]   = lhsT=eps pair (shared), rhs=ones8 [P,2,1]  (fp8 DR,
                   fused into the O loop -> its weight load is free)
    out          = (O + colsumP) * (1 / (rsum + S))   (DVE)
"""

from contextlib import ExitStack

import ml_dtypes
import numpy as np

import concourse.tile as tile
from concourse import bacc, mybir
from concourse.bass_utils import run_bass_kernel_spmd

F32 = mybir.dt.float32
BF16 = mybir.dt.bfloat16
F8 = mybir.dt.float8e4
NP_BF16 = ml_dtypes.bfloat16
NP_F8 = ml_dtypes.float8_e4m3fn
P = 128
WS = 16.0  # fp8 pre-scale for W_Q / W_K


def build_attention(nc, S=2048, D=1024, QB=1024, n_cores=8):
    """Emit the per-core attention program into `nc`. Requires S == 2*D == 2*QB."""
    assert S == 2 * D and QB == D and D % P == 0
    NST = S // P        # seq tiles (16)
    NDT = D // P        # d_model tiles (8)
    NQT = QB // P       # query tiles for this core (8)
    NPR = NDT // 2      # DoubleRow k-tile pairs, d_model contraction (4)
    NPS = NST // 2      # DoubleRow k-tile pairs, seq contraction (8)
    NC = min(512, D)    # matmul free-dim chunk (one PSUM bank of f32)
    NCH_D = D // NC     # chunks over output channels (2)
    NCH_Q = QB // NC    # chunks over queries (2)
    EXP = mybir.ActivationFunctionType.Exp
    DR = mybir.MatmulPerfMode.DoubleRow
    groups = [[2 * b, 2 * b + 1] for b in range(n_cores // 2)]

    xqt_ap = nc.dram_tensor("xqt", [P, NDT, QB], BF16, kind="ExternalInput").ap()
    xqt8_ap = nc.dram_tensor("xqt8", [P, NDT, QB], F8, kind="ExternalInput").ap()
    xpt8_ap = nc.dram_tensor("xpt8", [P, NDT, D], F8, kind="ExternalInput").ap()
    wqt_ap = nc.dram_tensor("wqt", [P, NDT, D], F8, kind="ExternalInput").ap()
    wkt_ap = nc.dram_tensor("wkt", [P, NDT, D], F8, kind="ExternalInput").ap()
    wvt_ap = nc.dram_tensor("wvt", [P, NDT, D], BF16, kind="ExternalInput").ap()
    out_ap = nc.dram_tensor("out", [QB, D], F32, kind="ExternalOutput").ap()

    with tile.TileContext(nc) as tc, ExitStack() as ctx:
        const_pool = ctx.enter_context(tc.tile_pool(name="const", bufs=1))
        qt_pool = ctx.enter_context(tc.tile_pool(name="qt", bufs=1))
        kr_pool = ctx.enter_context(tc.tile_pool(name="kr", bufs=1))
        v8_pool = ctx.enter_context(tc.tile_pool(name="v8", bufs=1))
        eps_pool = ctx.enter_context(tc.tile_pool(name="eps", bufs=1))
        dram = ctx.enter_context(tc.tile_pool(name="dram", bufs=1, space="DRAM"))
        psum_mm = ctx.enter_context(tc.tile_pool(name="psum_mm", bufs=6, space="PSUM"))

        ones = const_pool.tile([P, 1], BF16)        # colsum stationary
        nc.vector.memset(ones, 1.0)
        ones1 = const_pool.tile([1, P], BF16)       # K=1 broadcast stationary
        nc.vector.memset(ones1, 1.0)
        ones8 = const_pool.tile([P, 2, 1], F8)      # DR rowsum rhs
        nc.vector.memset(ones8, 1.0)

        QT8 = qt_pool.tile([P, NDT, QB], F8, name="QT8")
        KR8 = kr_pool.tile([P, NDT, S], F8, name="KR8")
        V8 = v8_pool.tile([P, NST, D], F8, name="V8")
        EPS = eps_pool.tile([P, NST, QB], F8, name="EPS")

        # DRAM bounce buffers, partition-major.  The V fragment has one
        # spare column block; its first two partition rows carry the
        # rank's exact bf16 colsum row (2*D fp8 bytes == D bf16 values).
        kr_frag = dram.tile([P, NDT * D], F8, name="kr_frag")
        kr_gath = dram.tile([2, P, NDT * D], F8, name="kr_gath")
        v_frag = dram.tile([P, (NQT + 1) * D], F8, name="v_frag")
        v_gath = dram.tile([2, P, (NQT + 1) * D], F8, name="v_gath")

        with tc.tile_pool(name="xt", bufs=1) as xt_pool, \
                tc.tile_pool(name="wt", bufs=1) as wt_pool, \
                tc.tile_pool(name="frag", bufs=2) as frag_pool, \
                tc.tile_pool(name="csp", bufs=1) as csp_pool, \
                tc.tile_pool(name="psum_cs", bufs=2, space="PSUM") as psum_cs:

            xpT8 = xt_pool.tile([P, NDT, D], F8, tag="xpT8", name="xpT8")
            wkT8 = wt_pool.tile([P, NDT, D], F8, tag="wkT8", name="wkT8")
            xqT = xt_pool.tile([P, NDT, QB], BF16, tag="xqT", name="xqT")
            xqT8 = xt_pool.tile([P, NDT, QB], F8, tag="xqT8", name="xqT8")
            wvT = wt_pool.tile([P, NDT, D], BF16, tag="wvT", name="wvT")
            wqT8 = wt_pool.tile([P, NDT, D], F8, tag="wqT8", name="wqT8")

            def load3d(src_ap, dst3, eng):
                # pre-tiled operand: 4 wide DMAs across queues
                for t in range(NPR):
                    eng.dma_start(out=dst3[:, 2 * t:2 * t + 2, :],
                                  in_=src_ap[:, 2 * t:2 * t + 2, :])

            # All operand loads are pushed before any data-dependent
            # descriptor: the DGE rings are in-order, so a paced fragment
            # write ahead of a load would stall that load's descriptor.
            # K-path operands (which gate the first matmul) go first.
            load3d(xpt8_ap, xpT8, nc.sync)
            load3d(wkt_ap, wkT8, nc.sync)
            load3d(xqt_ap, xqT, nc.scalar)
            load3d(wvt_ap, wvT, nc.scalar)
            load3d(xqt8_ap, xqT8, nc.sync)
            load3d(wqt_ap, wqT8, nc.sync)

            # ---- K_r half first: its AllGather hides under the V half ----
            for mt in range(NDT):
                kf = frag_pool.tile([P, D], F8, tag="kf", name="kf")
                pms = [psum_mm.tile([P, NC], F32, tag="pm", name=f"pm{i}") for i in range(NCH_D)]
                for t in range(NPR):
                    for cch in range(NCH_D):
                        nc.tensor.matmul(
                            pms[cch][:],
                            xpT8[:, 2 * t:2 * t + 2, mt * P:(mt + 1) * P],
                            wkT8[:, 2 * t:2 * t + 2, cch * NC:(cch + 1) * NC],
                            start=(t == 0), stop=(t == NPR - 1), perf_mode=DR,
                        )
                for cch in range(NCH_D):
                    nc.vector.tensor_copy(kf[:, cch * NC:(cch + 1) * NC], pms[cch][:])
                nc.scalar.dma_start(out=kr_frag[:, mt * D:(mt + 1) * D], in_=kf[:])
            nc.gpsimd.collective_compute(
                "AllGather", mybir.AluOpType.bypass, replica_groups=groups,
                ins=[kr_frag.opt()], outs=[kr_gath.opt()],
            )
            for g in range(2):
                nc.gpsimd.dma_start(
                    out=KR8[:, :, g * D:(g + 1) * D].opt(keep_dims={0, 1}),
                    in_=kr_gath[g],
                )

            # ---- V half: Vfrag[st] = xq @ W_V.T in bf16; fp8 for the
            # ---- gather plus exact bf16 colsum partial over this half.
            pcs = [psum_cs.tile([1, NC], F32, tag="pc", name=f"pc{i}") for i in range(NCH_D)]
            for st in range(NQT):
                vf8 = frag_pool.tile([P, D], F8, tag="vf8", name="vf8")
                vf16 = frag_pool.tile([P, D], BF16, tag="vf16", name="vf16", bufs=3)
                pms = [psum_mm.tile([P, NC], F32, tag="pm", name=f"pm{i}") for i in range(NCH_D)]
                for dt in range(NDT):
                    for cch in range(NCH_D):
                        nc.tensor.matmul(
                            pms[cch][:],
                            xqT[:, dt, st * P:(st + 1) * P],
                            wvT[:, dt, cch * NC:(cch + 1) * NC],
                            start=(dt == 0), stop=(dt == NDT - 1),
                        )
                for cch in range(NCH_D):
                    nc.vector.tensor_copy(vf8[:, cch * NC:(cch + 1) * NC], pms[cch][:])
                    nc.vector.tensor_copy(vf16[:, cch * NC:(cch + 1) * NC], pms[cch][:])
                    nc.tensor.matmul(
                        pcs[cch][:], ones[:], vf16[:, cch * NC:(cch + 1) * NC],
                        start=(st == 0), stop=(st == NQT - 1),
                    )
                nc.sync.dma_start(out=v_frag[:, st * D:(st + 1) * D], in_=vf8[:])
            # pack this half's bf16 colsum row into the spare column block
            cs_own = csp_pool.tile([1, D], BF16, name="cs_own")
            for cch in range(NCH_D):
                nc.scalar.copy(cs_own[:, cch * NC:(cch + 1) * NC], pcs[cch][:])
            nc.scalar.dma_start(
                out=v_frag[0:2, NQT * D:(NQT + 1) * D],
                in_=cs_own.bitcast(F8)[:],
            )
            nc.gpsimd.collective_compute(
                "AllGather", mybir.AluOpType.bypass, replica_groups=groups,
                ins=[v_frag.opt()], outs=[v_gath.opt()],
            )
            for g in range(2):
                nc.gpsimd.dma_start(
                    out=V8[:, g * NQT:(g + 1) * NQT, :].opt(keep_dims={0, 1}),
                    in_=v_gath[g][:, 0:NQT * D],
                )

            # ---- QT projection (fp8 DoubleRow; KR/V gathers in flight) ----
            for mt in range(NDT):
                pms = [psum_mm.tile([P, NC], F32, tag="pm", name=f"pm{i}") for i in range(NCH_Q)]
                for t in range(NPR):
                    for ich in range(NCH_Q):
                        nc.tensor.matmul(
                            pms[ich][:],
                            wqT8[:, 2 * t:2 * t + 2, mt * P:(mt + 1) * P],
                            xqT8[:, 2 * t:2 * t + 2, ich * NC:(ich + 1) * NC],
                            start=(t == 0), stop=(t == NPR - 1), perf_mode=DR,
                        )
                for ich in range(NCH_Q):
                    nc.scalar.copy(QT8[:, mt, ich * NC:(ich + 1) * NC], pms[ich][:])

        with tc.tile_pool(name="ets", bufs=4) as ets_pool, \
                tc.tile_pool(name="csum", bufs=1) as csum_pool, \
                tc.tile_pool(name="ostage", bufs=3) as ostage, \
                tc.tile_pool(name="recip", bufs=1) as recip_pool, \
                tc.tile_pool(name="psum_aux", bufs=1, space="PSUM") as psum_aux:

            # both halves' colsum rows -> add -> broadcast to 128 partitions
            cs_g = [csum_pool.tile([1, D], BF16, tag=f"cs{g}", name=f"cs{g}") for g in range(2)]
            for g in range(2):
                nc.gpsimd.dma_start(
                    out=cs_g[g].bitcast(F8)[:],
                    in_=v_gath[g][0:2, NQT * D:(NQT + 1) * D],
                )
            colsum = csum_pool.tile([1, D], BF16, name="colsum")
            nc.vector.tensor_tensor(colsum[:], cs_g[0][:], cs_g[1][:], mybir.AluOpType.add)
            colsumP = csum_pool.tile([P, D], F32, name="colsumP")
            for cch in range(NCH_D):
                pb = psum_aux.tile([P, NC], F32, tag="pc", name="pb")
                nc.tensor.matmul(pb[:], ones1[:], colsum[:, cch * NC:(cch + 1) * NC])
                nc.vector.tensor_copy(colsumP[:, cch * NC:(cch + 1) * NC], pb[:])

            # scores^T, exp, eps: EPS[:, jt, i] = exp(ST/(D*WS^2)) - 1 in fp8
            for jt in range(NST):
                pms = [psum_mm.tile([P, NC], F32, tag="pm", name=f"pm{i}") for i in range(NCH_Q)]
                for t in range(NPR):
                    for ich in range(NCH_Q):
                        nc.tensor.matmul(
                            pms[ich][:],
                            KR8[:, 2 * t:2 * t + 2, jt * P:(jt + 1) * P],
                            QT8[:, 2 * t:2 * t + 2, ich * NC:(ich + 1) * NC],
                            start=(t == 0), stop=(t == NPR - 1), perf_mode=DR,
                        )
                for ich in range(NCH_Q):
                    et = ets_pool.tile([P, NC], BF16, tag="et", name="et")
                    nc.scalar.activation(et[:], pms[ich][:], EXP, scale=1.0 / (D * WS * WS))
                    nc.vector.tensor_scalar_add(
                        EPS[:, jt, ich * NC:(ich + 1) * NC], et[:], -1.0
                    )

            # out[it][:, cch] = (colsumP + sum_j eps.T @ V8) / (S + sum_j eps)
            for it in range(NQT):
                pm0 = psum_mm.tile([P, NC], F32, tag="pm", name="pm0")
                pm1 = psum_mm.tile([P, NC], F32, tag="pm", name="pm1")
                pr = psum_aux.tile([P, 1], F32, tag="pr", name="pr")
                for t in range(NPS):
                    lhsT = EPS[:, 2 * t:2 * t + 2, it * P:(it + 1) * P]
                    nc.tensor.matmul(pm0[:], lhsT, V8[:, 2 * t:2 * t + 2, 0:NC],
                                     start=(t == 0), stop=(t == NPS - 1), perf_mode=DR)
                    nc.tensor.matmul(pm1[:], lhsT, V8[:, 2 * t:2 * t + 2, NC:2 * NC],
                                     start=(t == 0), stop=(t == NPS - 1), perf_mode=DR)
                    nc.tensor.matmul(pr[:], lhsT, ones8[:],
                                     start=(t == 0), stop=(t == NPS - 1), perf_mode=DR)
                rc = recip_pool.tile([P, 1], F32, tag="rc", name="rc", bufs=2)
                nc.vector.tensor_scalar_add(rc[:], pr[:], float(S))
                nc.vector.reciprocal(rc[:], rc[:])
                for cch, pm in ((0, pm0), (1, pm1)):
                    ob = ostage.tile([P, NC], F32, tag="ob", name="ob")
                    nc.vector.tensor_tensor(
                        ob[:], pm[:], colsumP[:, cch * NC:(cch + 1) * NC],
                        mybir.AluOpType.add,
                    )
                    nc.vector.tensor_scalar_mul(ob[:], ob[:], rc[:])
                    nc.sync.dma_start(
                        out=out_ap[it * P:(it + 1) * P, cch * NC:(cch + 1) * NC],
                        in_=ob[:],
                    )
    return nc


_CACHE = {}


def _get_nc(S=2048, D=1024, QB=1024):
    key = (S, D, QB)
    if key not in _CACHE:
        nc = bacc.Bacc("TRN2", target_bir_lowering=False, debug=False, num_devices=8)
        build_attention(nc, S=S, D=D, QB=QB, n_cores=8)
        nc.compile()
        _CACHE[key] = nc
    return _CACHE[key]


def _pack3d(a2d, np_dtype):
    # [D, cols] -> [P, NDT, cols]: contraction dim split as (tile, partition)
    d, cols = a2d.shape
    ndt = d // P
    return np.ascontiguousarray(
        a2d.reshape(ndt, P, cols).transpose(1, 0, 2).astype(np_dtype)
    )


def _run(x, W_Q, W_K, W_V, **spmd_kwargs):
    B, S, D = x.shape  # (4, 2048, 1024)
    QB = S // 2        # queries per core (1024)
    # host-side operand packing: everything pre-transposed (contraction on
    # DRAM rows) and pre-tiled [P, NDT, cols]; fp8e4 with x16 scale for the
    # Q/K path, bf16 for the V path
    x32 = np.asarray(x, dtype=np.float32)
    wqt = _pack3d(np.asarray(W_Q, dtype=np.float32).T * WS, NP_F8)
    wkt = _pack3d(np.asarray(W_K, dtype=np.float32).T * WS, NP_F8)
    wvt = _pack3d(np.asarray(W_V, dtype=np.float32).T, NP_BF16)
    ws = {"wqt": wqt, "wkt": wkt, "wvt": wvt}
    nc = _get_nc(S=S, D=D, QB=QB)
    in_maps = []
    for core in range(8):
        b, h = core // 2, core % 2
        xqt = x32[b, h * QB:(h + 1) * QB, :].T
        xpt = x32[b, h::2, :].T
        in_maps.append({
            "xqt": _pack3d(xqt, NP_BF16),
            "xqt8": _pack3d(xqt, NP_F8),
            "xpt8": _pack3d(xpt, NP_F8),
            **ws,
        })
    res = run_bass_kernel_spmd(nc, in_maps, list(range(8)), **spmd_kwargs)
    out = np.empty((B, S, D), dtype=np.float32)
    for core in range(8):
        b, h = core // 2, core % 2
        out[b, h * QB:(h + 1) * QB, :] = res.results[core]["out"]
    return out, res


def kernel(x, W_Q, W_K, W_V):
    return _run(x, W_Q, W_K, W_V)[0]
